# revision 96
# baseline (speedup 1.0000x reference)
"""FFTConv1d-with-threshold kernel for Trainium2, 8 NeuronCores.

Math: the reference (flat 16900-pt FFT -> prune coeffs with |Re|<0.01 ->
multiply by kernel FFT -> iFFT -> roll -> channel-sum -> slice) is
algebraically a standard 3x3 pad-1 conv2d applied to (xp - delta), where
delta is the inverse FFT of the pruned (below-threshold) coefficients.

Sharding: core = (batch b, input-channel half). Each core runs the FFT/
prune/inverse pipeline for its 16 channels and a partial 3x3 conv over
all 32 out-channels; the host sums the two partials per batch (+bias).

Device pipeline per core: 16900-pt FFT via Cooley-Tukey (100 x 169) as
fp32r PE matmuls, twiddle1 on DVE/Pool (fp32), prune mask, inverse
(fp32r/bf16) with twiddle2 applied post-transpose in bf16, subtract,
DRAM regather into 6 (tap-row, shift) replicated conv operand groups,
3x3 conv as 2 PSUM-accumulated matmuls per 4-row block.
"""

import ml_dtypes
import numpy as np

import bass_rust
import concourse.bass as bass
import concourse.mybir as mybir
from concourse.bass_utils import run_bass_kernel_spmd
from concourse.tile import TileContext

F32 = mybir.dt.float32
F32R = mybir.dt.float32r
BF16 = mybir.dt.bfloat16

N1, N2, N = 100, 169, 16900
W130 = 130
B, C, O = 4, 32, 32
S = C // 2           # channels per core (channel-split sharding)
THRESH = 0.01
FS1 = 338            # stage-1 F chunk: 2 channels x 169
NF1 = (S * N2) // FS1
FS2 = 400            # stage-2 F chunk: 4 channels x 100
NF2 = (S * N1) // FS2
QM = 85              # kept half-spectrum rows of the 169-DFT
QCH = [(0, 128), (128, 41)]
QS2 = 4736           # conv quarter span (cols), base 4160*g
XDPAD = 17536        # padded xdram row length (max read 4160*3+262+4735)
CA = 2863            # packed fp32r const blob cols
CH = 6412            # packed bf16 const blob cols


def _split_excess_waits(nc):
    # This walrus build accepts 1 sync-wait slot per instruction; Tile can
    # attach several. Move extras onto nofuse NOPs on the same engine.
    for f in nc.m.functions:
        for blk in f.blocks:
            insts = blk.instructions
            changed = False
            new_list = []
            for inst in insts:
                si = inst.sync_info
                if si is not None and len(si.on_wait) > 1:
                    waits = list(si.on_wait)
                    extra, keep = waits[:-1], waits[-1:]
                    for k, w in enumerate(extra):
                        new_list.append(bass_rust.InstNoOp(
                            name=f"{inst.name}-ws{k}",
                            engine=inst.engine,
                            ins=[], outs=[], bass_nofuse=True,
                            sync_info=bass_rust.SyncInfo(on_wait=[w], on_update=[]),
                        ))
                    inst.sync_info = bass_rust.SyncInfo(
                        on_wait=keep, on_update=list(si.on_update))
                    changed = True
                new_list.append(inst)
            if changed:
                blk.instructions = new_list


def _build():
    nc = bass.Bass("TRN2")
    x0 = nc.dram_tensor("x0", [N1, S * N2], F32R, kind="ExternalInput")
    wk = nc.dram_tensor("wk", [96, 64], BF16, kind="ExternalInput")
    blobA_d = nc.dram_tensor("blobA", [128, CA], F32R, kind="ExternalInput")
    blobH_d = nc.dram_tensor("blobH", [128, CH], BF16, kind="ExternalInput")
    out_part = nc.dram_tensor("out_part", [O, 128 * 128], F32, kind="ExternalOutput")
    xdram = nc.dram_tensor("xdram", [S, XDPAD], BF16)

    with TileContext(nc) as tc:
        with tc.tile_pool(name="const", bufs=1) as cst, \
             tc.tile_pool(name="big", bufs=1) as big, \
             tc.tile_pool(name="chunk", bufs=4) as chk:

            # packed constant blobs: slice map must match _consts()
            blobA = cst.tile([128, CA], F32R, tag="blobA", name="blobA")
            blobH = cst.tile([128, CH], BF16, tag="blobH", name="blobH")
            ct = {
                "c100c": blobA[0:N1, 0:100],
                "c100sn": blobA[0:N1, 100:200],
                "t1c2": blobA[0:N1, 200:538],
                "t1s2": blobA[0:N1, 538:876],
                "t1sn2": blobA[0:N1, 876:1214],
                "c169c": {0: blobA[0:128, 1342:1511], 128: blobA[0:41, 2187:2356]},
                "c169s": {0: blobA[0:128, 1511:1680], 128: blobA[0:41, 2356:2525]},
                "c169sn": {0: blobA[0:128, 2525:2694], 128: blobA[0:41, 2694:2863]},
                "c169cr": blobA[0:QM, 1680:1849],
                "c169sr": blobA[0:QM, 1849:2018],
                "c169snr": blobA[0:QM, 2018:2187],
                "w2c": blobH[0:N1, 0:2704],
                "w2s": blobH[0:N1, 2704:5408],
                "cinvc": blobH[0:N1, 5408:5508],
                "cinvsn": blobH[0:N1, 5508:5608],
                "c169ch": {0: blobH[0:128, 5736:5905], 128: blobH[0:41, 6074:6243]},
                "c169snh": {0: blobH[0:128, 5905:6074], 128: blobH[0:41, 6243:6412]},
            }
            ident = blobA[0:128, 1214:1342]
            identh = blobH[0:128, 5608:5736]
            wk_t = cst.tile([96, 64], BF16, tag="wk")

            x0t = big.tile([N1, S * N2], F32R, tag="x0")
            # load order = first-use order: x0 chunk 0, stage-1 consts,
            # rest of x0, stage-2/inverse consts, tw2 tables, weights
            nc.sync.dma_start(out=x0t[:, bass.ts(0, FS1)], in_=x0[:, bass.ts(0, FS1)])
            nc.sync.dma_start(out=blobA[0:N1, 0:200], in_=blobA_d[0:N1, 0:200])
            nc.sync.dma_start(out=blobA[0:N1, 200:1214], in_=blobA_d[0:N1, 200:1214])
            for f in range(1, NF1):
                sl = bass.ts(f, FS1)
                nc.sync.dma_start(out=x0t[:, sl], in_=x0[:, sl])
            nc.sync.dma_start(out=blobA[:, 1214:CA], in_=blobA_d[:, 1214:CA])
            nc.sync.dma_start(out=blobH[:, 5408:CH], in_=blobH_d[:, 5408:CH])
            nc.sync.dma_start(out=blobH[0:N1, 0:5408], in_=blobH_d[0:N1, 0:5408])
            nc.sync.dma_start(out=wk_t[:], in_=wk[:])
            xtr = big.tile([N1, S * N2], BF16, tag="xtr", name="xtr")
            # conv operand groups (tap-row r in {0,1}, shift s):
            # partition 16*(3r+s)+c col (g,k): x[c, 4160*g + 130r + s + k]
            xtq = big.tile([96, 4 * QS2], BF16, tag="xtq", name="xtq")

            htwtp_cm = tc.tile_pool(name="htwtp", bufs=1)
            htwtp = htwtp_cm.__enter__()
            # +676 col pad so batched T2 copies can use a rearrange window
            htwt_re = htwtp.tile([N1, S * N2 + 676], BF16, tag="htwt_re")
            htwt_im = htwtp.tile([N1, S * N2 + 676], BF16, tag="htwt_im")
            gttp_cm = tc.tile_pool(name="gttp", bufs=1)
            gttp = gttp_cm.__enter__()
            gtt_re_a = gttp.tile([128, S * N1], F32R, tag="gtt_re_a")
            gtt_re_b = gttp.tile([41, S * N1], F32R, tag="gtt_re_b")
            gtt_im_a = gttp.tile([128, S * N1], F32R, tag="gtt_im_a")
            gtt_im_b = gttp.tile([41, S * N1], F32R, tag="gtt_im_b")

            # ---------- FWD stage 1 + twiddle1 + T1 ----------
            with tc.tile_pool(name="ps1", bufs=2, space="PSUM") as ps1, \
                 tc.tile_pool(name="pt1", bufs=2, space="PSUM") as pt1:
                for f1 in range(NF1):
                    sl = bass.ts(f1, FS1)
                    ps_re = ps1.tile([N1, FS1], F32, tag="s1re")
                    ps_im = ps1.tile([N1, FS1], F32, tag="s1im")
                    nc.tensor.matmul(ps_re[:], ct["c100c"][:], x0t[:, sl], start=True, stop=True)
                    nc.tensor.matmul(ps_im[:], ct["c100sn"][:], x0t[:, sl], start=True, stop=True)
                    g_re = chk.tile([N1, FS1], F32R, tag="gt_re")
                    g_im = chk.tile([N1, FS1], F32R, tag="gt_im")
                    tmp1 = chk.tile([N1, FS1], F32, tag="tw1tmp")
                    tmp2 = chk.tile([N1, FS1], F32, tag="tw1tmp2")
                    tmp3 = chk.tile([N1, FS1], F32, tag="tw1tmp3")
                    tmp4 = chk.tile([N1, FS1], F32, tag="tw1tmp4")
                    # Gt_re = Gre*t1c + Gim*t1s ; Gt_im = Gim*t1c - Gre*t1s
                    nc.vector.tensor_mul(out=tmp1[:], in0=ps_re[:], in1=ct["t1c2"][:])
                    nc.vector.tensor_mul(out=tmp2[:], in0=ps_im[:], in1=ct["t1s2"][:])
                    nc.gpsimd.tensor_add(out=g_re[:], in0=tmp1[:], in1=tmp2[:])
                    nc.vector.tensor_mul(out=tmp3[:], in0=ps_im[:], in1=ct["t1c2"][:])
                    nc.vector.tensor_mul(out=tmp4[:], in0=ps_re[:], in1=ct["t1sn2"][:])
                    nc.gpsimd.tensor_add(out=g_im[:], in0=tmp3[:], in1=tmp4[:])
                    w2 = bass.ds(2 * f1 * N1, 2 * N1)
                    for g_src, dst_a, dst_b in (
                            (g_re, gtt_re_a, gtt_re_b),
                            (g_im, gtt_im_a, gtt_im_b)):
                        pt = pt1.tile([128, 512], F32R, tag="ptx")
                        for j in range(2):
                            nc.tensor.transpose(pt[0:128, bass.ts(j, N1)], g_src[:, bass.ds(j * N2, 128)], ident[0:N1, 0:N1])
                            nc.tensor.transpose(pt[0:41, bass.ds(200 + j * N1, N1)], g_src[:, bass.ds(j * N2 + 128, 41)], ident[0:N1, 0:N1])
                        nc.scalar.copy(out=dst_a[:, w2], in_=pt[0:128, 0:200])
                        nc.scalar.copy(out=dst_b[:, w2], in_=pt[0:41, 200:400])

            # ---- FWD stage 2 + mask + INV169 + T2 + tw2 + INV100 + store ----
            with tc.tile_pool(name="ps2", bufs=1, space="PSUM") as ps2, \
                 tc.tile_pool(name="pi1", bufs=2, space="PSUM") as pi1, \
                 tc.tile_pool(name="pt2", bufs=2, space="PSUM") as pt2, \
                 tc.tile_pool(name="ps3", bufs=1, space="PSUM") as ps3, \
                 tc.tile_pool(name="twp", bufs=3) as twp:
                for f in range(NF2):
                    sl = bass.ts(f, FS2)
                    ps_xre = ps2.tile([128, FS2], F32, tag="s2re")
                    ps_xim = ps2.tile([128, FS2], F32, tag="s2im")
                    lc_a = ct["c169c"][0][:, 0:QM]
                    lc_b = ct["c169c"][128][:, 0:QM]
                    ls_a = ct["c169s"][0][:, 0:QM]
                    ls_b = ct["c169s"][128][:, 0:QM]
                    # Xre = Cc.T@GtTre + Cs.T@GtTim   (fp32r)
                    nc.tensor.matmul(ps_xre[0:QM], lc_a, gtt_re_a[:, sl], start=True, stop=False)
                    nc.tensor.matmul(ps_xre[0:QM], lc_b, gtt_re_b[:, sl], start=False, stop=False)
                    nc.tensor.matmul(ps_xre[0:QM], ls_a, gtt_im_a[:, sl], start=False, stop=False)
                    nc.tensor.matmul(ps_xre[0:QM], ls_b, gtt_im_b[:, sl], start=False, stop=True)
                    # Xim = Cc.T@GtTim - Cs.T@GtTre   (fp32r)
                    nc.tensor.matmul(ps_xim[0:QM], ct["c169sn"][0][:, 0:QM], gtt_re_a[:, sl], start=True, stop=False)
                    nc.tensor.matmul(ps_xim[0:QM], ct["c169sn"][128][:, 0:QM], gtt_re_b[:, sl], start=False, stop=False)
                    nc.tensor.matmul(ps_xim[0:QM], ct["c169c"][0][:, 0:QM], gtt_im_a[:, sl], start=False, stop=False)
                    nc.tensor.matmul(ps_xim[0:QM], ct["c169c"][128][:, 0:QM], gtt_im_b[:, sl], start=False, stop=True)
                    ps_xre = ps_xre[0:QM]
                    ps_xim = ps_xim[0:QM]
                    pm = chk.tile([QM, FS2], F32, tag="pm")
                    nc.scalar.activation(pm[:], ps_xre[:],
                                         mybir.ActivationFunctionType.Abs)
                    zr = chk.tile([QM, FS2], F32R, tag="zre")
                    zi = chk.tile([QM, FS2], F32R, tag="zim")
                    # z = (|Xre| < t) * X   (fused mask-and-apply)
                    nc.vector.scalar_tensor_tensor(
                        out=zr[:], in0=pm[:], scalar=THRESH, in1=ps_xre[:],
                        op0=mybir.AluOpType.is_lt, op1=mybir.AluOpType.mult)
                    nc.vector.scalar_tensor_tensor(
                        out=zi[:], in0=pm[:], scalar=THRESH, in1=ps_xim[:],
                        op0=mybir.AluOpType.is_lt, op1=mybir.AluOpType.mult)

                    for (b0, bn) in QCH:
                        ps_hre = pi1.tile([128, FS2], F32, tag="i1re")
                        ps_him = pi1.tile([128, FS2], F32, tag="i1im", bufs=1)
                        lc = ct["c169cr"][:, bass.ds(b0, bn)]
                        ls = ct["c169sr"][:, bass.ds(b0, bn)]
                        lsn = ct["c169snr"][:, bass.ds(b0, bn)]
                        # Hre = Cc.T@Zre - Cs.T@Zim ; Him = Cs.T@Zre + Cc.T@Zim
                        nc.tensor.matmul(ps_hre[0:bn], lc, zr[:], start=True, stop=False)
                        nc.tensor.matmul(ps_hre[0:bn], lsn, zi[:], start=False, stop=True)
                        nc.tensor.matmul(ps_him[0:bn], ls, zr[:], start=True, stop=False)
                        nc.tensor.matmul(ps_him[0:bn], lc, zi[:], start=False, stop=True)
                        h_re = chk.tile([128, FS2], BF16, tag="h_re")
                        h_im = chk.tile([128, FS2], BF16, tag="h_im")
                        nc.scalar.copy(out=h_re[0:bn], in_=ps_hre[0:bn])
                        nc.vector.tensor_copy(h_im[0:bn], ps_him[0:bn])
                        for src, dst in ((h_re, htwt_re), (h_im, htwt_im)):
                            pt = pt2.tile([N1, 512], BF16, tag="t2p")
                            for j in range(4):
                                nc.tensor.transpose(pt[:, bass.ds(j * 128, bn)], src[0:bn, bass.ts(j, N1)], identh[0:bn, 0:bn])
                            nc.scalar.copy(
                                out=dst[:, bass.ds(4 * f * N2 + b0, 676)]
                                    .rearrange("p (j b) -> p j b", b=N2)[:, :, 0:bn],
                                in_=pt[:].rearrange("p (j b) -> p j b", b=128)[:, :, 0:bn])

                    # twiddle2 for this chunk's 4 channels, [k1,(c,n2)] bf16:
                    # Htw = H * (w2c + i*w2s): re = re*c - im*s ; im = re*s + im*c
                    us = bass.ds(f * 4 * N2, 4 * N2)
                    ta = twp.tile([N1, 4 * N2], BF16, tag="tta", name="tta")
                    tb = twp.tile([N1, 4 * N2], BF16, tag="ttb", name="ttb")
                    nc.gpsimd.tensor_mul(out=ta[:], in0=htwt_re[:, us], in1=ct["w2s"][:, us])
                    nc.vector.tensor_mul(out=tb[:], in0=htwt_im[:, us], in1=ct["w2s"][:, us])
                    nc.vector.tensor_mul(out=htwt_re[:, us], in0=htwt_re[:, us], in1=ct["w2c"][:, us])
                    nc.vector.tensor_mul(out=htwt_im[:, us], in0=htwt_im[:, us], in1=ct["w2c"][:, us])
                    nc.vector.tensor_sub(out=htwt_re[:, us], in0=htwt_re[:, us], in1=tb[:])
                    nc.vector.tensor_add(out=htwt_im[:, us], in0=htwt_im[:, us], in1=ta[:])

                    # INV stage 2 fused with subtract: psum = I@x0 - cinv@Htw
                    # (cinvc/cinvsn are pre-negated on the host)
                    for f1 in (2 * f, 2 * f + 1):
                        sl1 = bass.ts(f1, FS1)
                        ps_d = ps3.tile([N1, FS1], F32, tag="dlt")
                        nc.tensor.matmul(ps_d[:], ident[0:N1, 0:N1], x0t[:, sl1], start=True, stop=False)
                        nc.tensor.matmul(ps_d[:], ct["cinvc"][:], htwt_re[:, sl1], start=False, stop=False)
                        nc.tensor.matmul(ps_d[:], ct["cinvsn"][:], htwt_im[:, sl1], start=False, stop=True)
                        nc.scalar.copy(out=xtr[:, sl1], in_=ps_d[:])
                    # flatten this chunk's 4 channels to DRAM [c, 16900] (bf16)
                    nc.gpsimd.dma_start(
                        out=xdram[bass.ds(4 * f, 4), 0:N]
                            .rearrange("c (a b) -> a c b", b=N2),
                        in_=xtr[:, bass.ds(f * 4 * N2, 4 * N2)])
                    # conv operand sub-gathers, every 2 chunks (8 channels)
                    if f % 2 == 1:
                        c8 = 4 * (f - 1)
                        for r in range(2):
                            for s in range(3):
                                grp = r * 3 + s
                                src = bass.AP(
                                    tensor=xdram, offset=c8 * XDPAD + 130 * r + s,
                                    ap=[[XDPAD, 8], [4160, 4], [1, QS2]])
                                nc.sync.dma_start(
                                    out=xtq[bass.ds(16 * grp + c8, 8), :]
                                        .rearrange("c (g k) -> c g k", k=QS2),
                                    in_=src)

            gttp_cm.__exit__(None, None, None)
            htwtp_cm.__exit__(None, None, None)

            # ---------- conv 3x3 valid: 2 matmuls per 4-row block ----------
            with tc.tile_pool(name="psc", bufs=6, space="PSUM") as psc, \
                 tc.tile_pool(name="pswm", bufs=1, space="PSUM") as pswm, \
                 tc.tile_pool(name="ostp", bufs=4) as ostp:
                # keep the PE clock ramped through the gather gap: a chain of
                # throwaway matmuls on resident data (result never read)
                ps_w = pswm.tile([32, 512], F32, tag="warm", name="ps_w")
                for _ in range(29):
                    nc.tensor.matmul(ps_w[:], ident[0:N1, 0:32],
                                     x0t[:, 0:512], start=True, stop=True)
                for q in range(8):
                    ost = ostp.tile([O, 2048], F32, tag="ost", name="ost")
                    for pp in range(4):
                        p = 4 * q + pp
                        g, lp = p // 8, p % 8
                        off0 = 520 * lp + QS2 * g
                        ps_o = psc.tile([O, 512], F32, tag="conv")
                        rhsA = xtq[0:96, off0:off0 + 520] \
                            .rearrange("c (i w) -> c i w", w=W130)[:, :, 0:128]
                        rhsB = xtq[0:48, off0 + 260:off0 + 780] \
                            .rearrange("c (i w) -> c i w", w=W130)[:, :, 0:128]
                        nc.tensor.matmul(ps_o[:].rearrange("o (i t) -> o i t", t=128),
                                         wk_t[0:96, 0:O], rhsA, start=True, stop=False)
                        nc.tensor.matmul(ps_o[:].rearrange("o (i t) -> o i t", t=128),
                                         wk_t[0:48, 32:32 + O], rhsB, start=False, stop=True)
                        if pp % 2 == 0:
                            nc.scalar.copy(out=ost[:, bass.ts(pp, 512)], in_=ps_o[:])
                        else:
                            nc.vector.tensor_copy(ost[:, bass.ts(pp, 512)], ps_o[:])
                    nc.sync.dma_start(out=out_part[:, bass.ts(q, 2048)], in_=ost[:])

    _split_excess_waits(nc)
    return nc


_NC_CACHE = {}


def _get_nc():
    if "nc" not in _NC_CACHE:
        _NC_CACHE["nc"] = _build()
    return _NC_CACHE["nc"]


def _consts():
    if "consts" in _NC_CACHE:
        return _NC_CACHE["consts"]
    import ml_dtypes
    r = np.arange(N1)
    q = np.arange(N2)
    a100 = 2 * np.pi * np.outer(r, r) / N1
    a169 = 2 * np.pi * np.outer(q, q) / N2
    t1 = 2 * np.pi * np.outer(r, q) / N       # [r, b]
    c169c, c169s = np.cos(a169), np.sin(a169)

    def put(blob, r0, c0, v):
        blob[r0:r0 + v.shape[0], c0:c0 + v.shape[1]] = v

    blobA = np.zeros((128, CA), dtype=np.float32)
    put(blobA, 0, 0, np.cos(a100))
    put(blobA, 0, 100, -np.sin(a100))
    put(blobA, 0, 200, np.tile(np.cos(t1), (1, 2)))
    put(blobA, 0, 538, np.tile(np.sin(t1), (1, 2)))
    put(blobA, 0, 876, np.tile(-np.sin(t1), (1, 2)))
    put(blobA, 0, 1214, np.eye(128))
    put(blobA, 0, 1342, c169c[0:128])
    put(blobA, 0, 1511, c169s[0:128])
    # inverse 169-DFT half-spectrum rows, conjugate doubling (x2) folded in.
    # (drops the exact w=1 at DC(0,0) and the q=84 row: ~3e-4 rel error)
    put(blobA, 0, 1680, 2.0 * c169c[0:QM])
    put(blobA, 0, 1849, 2.0 * c169s[0:QM])
    put(blobA, 0, 2018, -2.0 * c169s[0:QM])
    put(blobA, 0, 2187, c169c[128:N2])
    put(blobA, 0, 2356, c169s[128:N2])
    put(blobA, 0, 2525, -c169s[0:128])
    put(blobA, 0, 2694, -c169s[128:N2])

    blobH = np.zeros((128, CH), dtype=np.float32)
    put(blobH, 0, 0, np.tile(np.cos(t1), (1, S)))
    put(blobH, 0, 2704, np.tile(np.sin(t1), (1, S)))
    put(blobH, 0, 5408, -np.cos(a100) / N)
    put(blobH, 0, 5508, np.sin(a100) / N)
    put(blobH, 0, 5608, np.eye(128))
    put(blobH, 0, 5736, c169c[0:128])
    put(blobH, 0, 5905, -c169s[0:128])
    put(blobH, 0, 6074, c169c[128:N2])
    put(blobH, 0, 6243, -c169s[128:N2])

    cc = {"blobA": blobA,
          "blobH": blobH.astype(ml_dtypes.bfloat16)}
    _NC_CACHE["consts"] = cc
    return cc


def kernel(x, weight, bias):
    x = np.asarray(x, dtype=np.float32)
    weight = np.asarray(weight, dtype=np.float32)
    bias = np.asarray(bias, dtype=np.float32)
    nc = _get_nc()
    cc = _consts()

    xp = np.pad(x, ((0, 0), (0, 0), (1, 1), (1, 1)))          # (4,32,130,130)
    # [a, (c, b)] layout of the flat 16900 signal, per batch
    x0s = [np.ascontiguousarray(
        xp[b].reshape(C, N).reshape(C, N1, N2).transpose(1, 0, 2).reshape(N1, C * N2))
        for b in range(B)]

    in_maps = []
    for core in range(8):
        b, h = core // 2, core % 2
        c0 = h * S
        # wk[:, 0:32]: rows (r in {0,1}, s, c) -> weight[o, c, r, s]
        # wk[0:48, 32:64]: rows (s, c) -> weight[o, c, 2, s]  (tap row r=2)
        wkm = np.zeros((96, 64), dtype=np.float32)
        for rr in range(2):
            for s in range(3):
                wkm[rr * 48 + s * 16:rr * 48 + s * 16 + 16, 0:32] = \
                    weight[:, c0:c0 + S, rr, s].T
        for s in range(3):
            wkm[s * 16:s * 16 + 16, 32:64] = weight[:, c0:c0 + S, 2, s].T
        m = {"x0": np.ascontiguousarray(x0s[b][:, c0 * N2:(c0 + S) * N2]),
             "wk": wkm.astype(ml_dtypes.bfloat16)}
        m.update(cc)
        in_maps.append(m)

    res = run_bass_kernel_spmd(nc, in_maps, core_ids=list(range(8)))

    out = np.empty((B, O, 128, 128), dtype=np.float32)
    for b in range(B):
        acc = res.results[2 * b]["out_part"] + res.results[2 * b + 1]["out_part"]
        out[b] = acc.reshape(O, 128, 128)
    out += bias[None, :, None, None]
    return out


# revision 97
# speedup vs baseline: 1.0442x; 1.0442x over previous
"""FFTConv1d-with-threshold kernel for Trainium2, 8 NeuronCores.

Math: the reference (flat 16900-pt FFT -> prune coeffs with |Re|<0.01 ->
multiply by kernel FFT -> iFFT -> roll -> channel-sum -> slice) is
algebraically a standard 3x3 pad-1 conv2d applied to (xp - delta), where
delta is the inverse FFT of the pruned (below-threshold) coefficients.

Sharding: core = (batch b, input-channel half). Each core runs the FFT/
prune/inverse pipeline for its 16 channels and a partial 3x3 conv over
all 32 out-channels; the host sums the two partials per batch (+bias).

Device pipeline per core: 16900-pt FFT via Cooley-Tukey (100 x 169) as
fp32r PE matmuls, twiddle1 on DVE/Pool (fp32), prune mask, inverse
(fp32r/bf16) with twiddle2 applied post-transpose in bf16, subtract,
DRAM regather into 6 (tap-row, shift) replicated conv operand groups,
3x3 conv as 2 PSUM-accumulated matmuls per 4-row block.
"""

import ml_dtypes
import numpy as np

import bass_rust
import concourse.bass as bass
import concourse.mybir as mybir
from concourse.bass_utils import run_bass_kernel_spmd
from concourse.tile import TileContext

F32 = mybir.dt.float32
F32R = mybir.dt.float32r
BF16 = mybir.dt.bfloat16

N1, N2, N = 100, 169, 16900
W130 = 130
B, C, O = 4, 32, 32
S = C // 2           # channels per core (channel-split sharding)
THRESH = 0.01
FS1 = 338            # stage-1 F chunk: 2 channels x 169
NF1 = (S * N2) // FS1
FS2 = 400            # stage-2 F chunk: 4 channels x 100
NF2 = (S * N1) // FS2
QM = 85              # kept half-spectrum rows of the 169-DFT
QCH = [(0, 128), (128, 41)]
QS2 = 4736           # conv quarter span (cols), base 4160*g
XDPAD = 17536        # padded xdram row length (max read 4160*3+262+4735)
CA = 2863            # packed fp32r const blob cols
CH = 6412            # packed bf16 const blob cols


def _split_excess_waits(nc):
    # This walrus build accepts 1 sync-wait slot per instruction; Tile can
    # attach several. Move extras onto nofuse NOPs on the same engine.
    for f in nc.m.functions:
        for blk in f.blocks:
            insts = blk.instructions
            changed = False
            new_list = []
            for inst in insts:
                si = inst.sync_info
                if si is not None and len(si.on_wait) > 1:
                    waits = list(si.on_wait)
                    extra, keep = waits[:-1], waits[-1:]
                    for k, w in enumerate(extra):
                        new_list.append(bass_rust.InstNoOp(
                            name=f"{inst.name}-ws{k}",
                            engine=inst.engine,
                            ins=[], outs=[], bass_nofuse=True,
                            sync_info=bass_rust.SyncInfo(on_wait=[w], on_update=[]),
                        ))
                    inst.sync_info = bass_rust.SyncInfo(
                        on_wait=keep, on_update=list(si.on_update))
                    changed = True
                new_list.append(inst)
            if changed:
                blk.instructions = new_list


def _build():
    nc = bass.Bass("TRN2")
    x0 = nc.dram_tensor("x0", [N1, S * N2], F32R, kind="ExternalInput")
    wk = nc.dram_tensor("wk", [96, 64], BF16, kind="ExternalInput")
    blobA_d = nc.dram_tensor("blobA", [128, CA], F32R, kind="ExternalInput")
    blobH_d = nc.dram_tensor("blobH", [128, CH], BF16, kind="ExternalInput")
    out_part = nc.dram_tensor("out_part", [O, 128 * 128], F32, kind="ExternalOutput")
    xdram = nc.dram_tensor("xdram", [S, XDPAD], BF16)

    with TileContext(nc) as tc:
        with tc.tile_pool(name="const", bufs=1) as cst, \
             tc.tile_pool(name="big", bufs=1) as big, \
             tc.tile_pool(name="chunk", bufs=4) as chk:

            # packed constant blobs: slice map must match _consts()
            blobA = cst.tile([128, CA], F32R, tag="blobA", name="blobA")
            blobH = cst.tile([128, CH], BF16, tag="blobH", name="blobH")
            ct = {
                "c100c": blobA[0:N1, 0:100],
                "c100sn": blobA[0:N1, 100:200],
                "t1c2": blobA[0:N1, 200:538],
                "t1s2": blobA[0:N1, 538:876],
                "t1sn2": blobA[0:N1, 876:1214],
                "c169c": {0: blobA[0:128, 1342:1511], 128: blobA[0:41, 2187:2356]},
                "c169s": {0: blobA[0:128, 1511:1680], 128: blobA[0:41, 2356:2525]},
                "c169sn": {0: blobA[0:128, 2525:2694], 128: blobA[0:41, 2694:2863]},
                "c169cr": blobA[0:QM, 1680:1849],
                "c169sr": blobA[0:QM, 1849:2018],
                "c169snr": blobA[0:QM, 2018:2187],
                "w2c": blobH[0:N1, 0:2704],
                "w2s": blobH[0:N1, 2704:5408],
                "cinvc": blobH[0:N1, 5408:5508],
                "cinvsn": blobH[0:N1, 5508:5608],
                "c169ch": {0: blobH[0:128, 5736:5905], 128: blobH[0:41, 6074:6243]},
                "c169snh": {0: blobH[0:128, 5905:6074], 128: blobH[0:41, 6243:6412]},
            }
            ident = blobA[0:128, 1214:1342]
            identh = blobH[0:128, 5608:5736]
            wk_t = cst.tile([96, 64], BF16, tag="wk")

            x0t = big.tile([N1, S * N2], F32R, tag="x0")
            # load order = first-use order: x0 chunk 0, stage-1 consts,
            # rest of x0, stage-2/inverse consts, tw2 tables, weights
            nc.sync.dma_start(out=x0t[:, bass.ts(0, FS1)], in_=x0[:, bass.ts(0, FS1)])
            nc.sync.dma_start(out=blobA[0:N1, 0:200], in_=blobA_d[0:N1, 0:200])
            nc.sync.dma_start(out=blobA[0:N1, 200:1214], in_=blobA_d[0:N1, 200:1214])
            for f in range(1, NF1):
                sl = bass.ts(f, FS1)
                nc.sync.dma_start(out=x0t[:, sl], in_=x0[:, sl])
            nc.sync.dma_start(out=blobA[:, 1214:CA], in_=blobA_d[:, 1214:CA])
            nc.sync.dma_start(out=blobH[:, 5408:CH], in_=blobH_d[:, 5408:CH])
            nc.sync.dma_start(out=blobH[0:N1, 0:5408], in_=blobH_d[0:N1, 0:5408])
            nc.sync.dma_start(out=wk_t[:], in_=wk[:])
            xtr = big.tile([N1, S * N2], BF16, tag="xtr", name="xtr")
            # conv operand groups (tap-row r in {0,1}, shift s):
            # partition 16*(3r+s)+c col (g,k): x[c, 4160*g + 130r + s + k]
            xtq = big.tile([96, 4 * QS2], BF16, tag="xtq", name="xtq")

            htwtp_cm = tc.tile_pool(name="htwtp", bufs=1)
            htwtp = htwtp_cm.__enter__()
            # +676 col pad so batched T2 copies can use a rearrange window
            htwt_re = htwtp.tile([N1, S * N2 + 676], BF16, tag="htwt_re")
            htwt_im = htwtp.tile([N1, S * N2 + 676], BF16, tag="htwt_im")
            gttp_cm = tc.tile_pool(name="gttp", bufs=1)
            gttp = gttp_cm.__enter__()
            gtt_re_a = gttp.tile([128, S * N1], F32R, tag="gtt_re_a")
            gtt_re_b = gttp.tile([41, S * N1], F32R, tag="gtt_re_b")
            gtt_im_a = gttp.tile([128, S * N1], F32R, tag="gtt_im_a")
            gtt_im_b = gttp.tile([41, S * N1], F32R, tag="gtt_im_b")

            # ---------- FWD stage 1 + twiddle1 + T1 ----------
            with tc.tile_pool(name="ps1", bufs=2, space="PSUM") as ps1, \
                 tc.tile_pool(name="pt1", bufs=4, space="PSUM") as pt1:
                for f1 in range(NF1):
                    sl = bass.ts(f1, FS1)
                    ps_re = ps1.tile([N1, FS1], F32, tag="s1re")
                    ps_im = ps1.tile([N1, FS1], F32, tag="s1im")
                    nc.tensor.matmul(ps_re[:], ct["c100c"][:], x0t[:, sl], start=True, stop=True)
                    nc.tensor.matmul(ps_im[:], ct["c100sn"][:], x0t[:, sl], start=True, stop=True)
                    g_re = chk.tile([N1, FS1], F32R, tag="gt_re")
                    g_im = chk.tile([N1, FS1], F32R, tag="gt_im")
                    tmp1 = chk.tile([N1, FS1], F32, tag="tw1tmp")
                    tmp2 = chk.tile([N1, FS1], F32, tag="tw1tmp2")
                    tmp3 = chk.tile([N1, FS1], F32, tag="tw1tmp3")
                    tmp4 = chk.tile([N1, FS1], F32, tag="tw1tmp4")
                    # Gt_re = Gre*t1c + Gim*t1s ; Gt_im = Gim*t1c - Gre*t1s
                    nc.vector.tensor_mul(out=tmp1[:], in0=ps_re[:], in1=ct["t1c2"][:])
                    nc.vector.tensor_mul(out=tmp2[:], in0=ps_im[:], in1=ct["t1s2"][:])
                    nc.gpsimd.tensor_add(out=g_re[:], in0=tmp1[:], in1=tmp2[:])
                    nc.vector.tensor_mul(out=tmp3[:], in0=ps_im[:], in1=ct["t1c2"][:])
                    nc.vector.tensor_mul(out=tmp4[:], in0=ps_re[:], in1=ct["t1sn2"][:])
                    nc.gpsimd.tensor_add(out=g_im[:], in0=tmp3[:], in1=tmp4[:])
                    w2 = bass.ds(2 * f1 * N1, 2 * N1)
                    for g_src, dst_a, dst_b in (
                            (g_re, gtt_re_a, gtt_re_b),
                            (g_im, gtt_im_a, gtt_im_b)):
                        pt = pt1.tile([128, 512], F32R, tag="ptx")
                        for j in range(2):
                            nc.tensor.transpose(pt[0:128, bass.ts(j, N1)], g_src[:, bass.ds(j * N2, 128)], ident[0:N1, 0:N1])
                            nc.tensor.transpose(pt[0:41, bass.ds(200 + j * N1, N1)], g_src[:, bass.ds(j * N2 + 128, 41)], ident[0:N1, 0:N1])
                        nc.scalar.copy(out=dst_a[:, w2], in_=pt[0:128, 0:200])
                        nc.scalar.copy(out=dst_b[:, w2], in_=pt[0:41, 200:400])

            # ---- FWD stage 2 + mask + INV169 + T2 + tw2 + INV100 + store ----
            with tc.tile_pool(name="ps2", bufs=1, space="PSUM") as ps2, \
                 tc.tile_pool(name="pi1", bufs=2, space="PSUM") as pi1, \
                 tc.tile_pool(name="pt2", bufs=2, space="PSUM") as pt2, \
                 tc.tile_pool(name="ps3", bufs=1, space="PSUM") as ps3, \
                 tc.tile_pool(name="twp", bufs=3) as twp:
                for f in range(NF2):
                    sl = bass.ts(f, FS2)
                    ps_xre = ps2.tile([128, FS2], F32, tag="s2re")
                    ps_xim = ps2.tile([128, FS2], F32, tag="s2im")
                    lc_a = ct["c169c"][0][:, 0:QM]
                    lc_b = ct["c169c"][128][:, 0:QM]
                    ls_a = ct["c169s"][0][:, 0:QM]
                    ls_b = ct["c169s"][128][:, 0:QM]
                    # Xre = Cc.T@GtTre + Cs.T@GtTim   (fp32r)
                    nc.tensor.matmul(ps_xre[0:QM], lc_a, gtt_re_a[:, sl], start=True, stop=False)
                    nc.tensor.matmul(ps_xre[0:QM], lc_b, gtt_re_b[:, sl], start=False, stop=False)
                    nc.tensor.matmul(ps_xre[0:QM], ls_a, gtt_im_a[:, sl], start=False, stop=False)
                    nc.tensor.matmul(ps_xre[0:QM], ls_b, gtt_im_b[:, sl], start=False, stop=True)
                    # Xim = Cc.T@GtTim - Cs.T@GtTre   (fp32r)
                    nc.tensor.matmul(ps_xim[0:QM], ct["c169sn"][0][:, 0:QM], gtt_re_a[:, sl], start=True, stop=False)
                    nc.tensor.matmul(ps_xim[0:QM], ct["c169sn"][128][:, 0:QM], gtt_re_b[:, sl], start=False, stop=False)
                    nc.tensor.matmul(ps_xim[0:QM], ct["c169c"][0][:, 0:QM], gtt_im_a[:, sl], start=False, stop=False)
                    nc.tensor.matmul(ps_xim[0:QM], ct["c169c"][128][:, 0:QM], gtt_im_b[:, sl], start=False, stop=True)
                    ps_xre = ps_xre[0:QM]
                    ps_xim = ps_xim[0:QM]
                    pm = chk.tile([QM, FS2], F32, tag="pm")
                    nc.scalar.activation(pm[:], ps_xre[:],
                                         mybir.ActivationFunctionType.Abs)
                    zr = chk.tile([QM, FS2], F32R, tag="zre")
                    zi = chk.tile([QM, FS2], F32R, tag="zim")
                    # z = (|Xre| < t) * X   (fused mask-and-apply)
                    nc.vector.scalar_tensor_tensor(
                        out=zr[:], in0=pm[:], scalar=THRESH, in1=ps_xre[:],
                        op0=mybir.AluOpType.is_lt, op1=mybir.AluOpType.mult)
                    nc.vector.scalar_tensor_tensor(
                        out=zi[:], in0=pm[:], scalar=THRESH, in1=ps_xim[:],
                        op0=mybir.AluOpType.is_lt, op1=mybir.AluOpType.mult)

                    for (b0, bn) in QCH:
                        ps_hre = pi1.tile([128, FS2], F32, tag="i1re")
                        ps_him = pi1.tile([128, FS2], F32, tag="i1im", bufs=1)
                        lc = ct["c169cr"][:, bass.ds(b0, bn)]
                        ls = ct["c169sr"][:, bass.ds(b0, bn)]
                        lsn = ct["c169snr"][:, bass.ds(b0, bn)]
                        # Hre = Cc.T@Zre - Cs.T@Zim ; Him = Cs.T@Zre + Cc.T@Zim
                        nc.tensor.matmul(ps_hre[0:bn], lc, zr[:], start=True, stop=False)
                        nc.tensor.matmul(ps_hre[0:bn], lsn, zi[:], start=False, stop=True)
                        nc.tensor.matmul(ps_him[0:bn], ls, zr[:], start=True, stop=False)
                        nc.tensor.matmul(ps_him[0:bn], lc, zi[:], start=False, stop=True)
                        h_re = chk.tile([128, FS2], BF16, tag="h_re")
                        h_im = chk.tile([128, FS2], BF16, tag="h_im")
                        nc.scalar.copy(out=h_re[0:bn], in_=ps_hre[0:bn])
                        nc.vector.tensor_copy(h_im[0:bn], ps_him[0:bn])
                        for src, dst in ((h_re, htwt_re), (h_im, htwt_im)):
                            pt = pt2.tile([N1, 512], BF16, tag="t2p")
                            for j in range(4):
                                nc.tensor.transpose(pt[:, bass.ds(j * 128, bn)], src[0:bn, bass.ts(j, N1)], identh[0:bn, 0:bn])
                            nc.scalar.copy(
                                out=dst[:, bass.ds(4 * f * N2 + b0, 676)]
                                    .rearrange("p (j b) -> p j b", b=N2)[:, :, 0:bn],
                                in_=pt[:].rearrange("p (j b) -> p j b", b=128)[:, :, 0:bn])

                    # twiddle2 for this chunk's 4 channels, [k1,(c,n2)] bf16:
                    # Htw = H * (w2c + i*w2s): re = re*c - im*s ; im = re*s + im*c
                    us = bass.ds(f * 4 * N2, 4 * N2)
                    ta = twp.tile([N1, 4 * N2], BF16, tag="tta", name="tta")
                    tb = twp.tile([N1, 4 * N2], BF16, tag="ttb", name="ttb")
                    nc.gpsimd.tensor_mul(out=ta[:], in0=htwt_re[:, us], in1=ct["w2s"][:, us])
                    nc.vector.tensor_mul(out=tb[:], in0=htwt_im[:, us], in1=ct["w2s"][:, us])
                    nc.vector.tensor_mul(out=htwt_re[:, us], in0=htwt_re[:, us], in1=ct["w2c"][:, us])
                    nc.vector.tensor_mul(out=htwt_im[:, us], in0=htwt_im[:, us], in1=ct["w2c"][:, us])
                    nc.vector.tensor_sub(out=htwt_re[:, us], in0=htwt_re[:, us], in1=tb[:])
                    nc.vector.tensor_add(out=htwt_im[:, us], in0=htwt_im[:, us], in1=ta[:])

                    # INV stage 2 fused with subtract: psum = I@x0 - cinv@Htw
                    # (cinvc/cinvsn are pre-negated on the host)
                    for f1 in (2 * f, 2 * f + 1):
                        sl1 = bass.ts(f1, FS1)
                        ps_d = ps3.tile([N1, FS1], F32, tag="dlt")
                        nc.tensor.matmul(ps_d[:], ident[0:N1, 0:N1], x0t[:, sl1], start=True, stop=False)
                        nc.tensor.matmul(ps_d[:], ct["cinvc"][:], htwt_re[:, sl1], start=False, stop=False)
                        nc.tensor.matmul(ps_d[:], ct["cinvsn"][:], htwt_im[:, sl1], start=False, stop=True)
                        nc.scalar.copy(out=xtr[:, sl1], in_=ps_d[:])
                    # flatten this chunk's 4 channels to DRAM [c, 16900] (bf16)
                    nc.gpsimd.dma_start(
                        out=xdram[bass.ds(4 * f, 4), 0:N]
                            .rearrange("c (a b) -> a c b", b=N2),
                        in_=xtr[:, bass.ds(f * 4 * N2, 4 * N2)])
                    # conv operand sub-gathers for this chunk's 4 channels
                    for r in range(2):
                        for s in range(3):
                            grp = r * 3 + s
                            src = bass.AP(
                                tensor=xdram, offset=4 * f * XDPAD + 130 * r + s,
                                ap=[[XDPAD, 4], [4160, 4], [1, QS2]])
                            nc.sync.dma_start(
                                out=xtq[bass.ds(16 * grp + 4 * f, 4), :]
                                    .rearrange("c (g k) -> c g k", k=QS2),
                                in_=src)

            gttp_cm.__exit__(None, None, None)
            htwtp_cm.__exit__(None, None, None)

            # ---------- conv 3x3 valid: 2 matmuls per 4-row block ----------
            with tc.tile_pool(name="psc", bufs=6, space="PSUM") as psc, \
                 tc.tile_pool(name="pswm", bufs=1, space="PSUM") as pswm, \
                 tc.tile_pool(name="ostp", bufs=4) as ostp:
                # keep the PE clock ramped through the gather gap: a chain of
                # throwaway matmuls on resident data (result never read)
                ps_w = pswm.tile([32, 512], F32, tag="warm", name="ps_w")
                for _ in range(29):
                    nc.tensor.matmul(ps_w[:], ident[0:N1, 0:32],
                                     x0t[:, 0:512], start=True, stop=True)
                for q in range(8):
                    ost = ostp.tile([O, 2048], F32, tag="ost", name="ost")
                    for pp in range(4):
                        p = 4 * q + pp
                        g, lp = p // 8, p % 8
                        off0 = 520 * lp + QS2 * g
                        ps_o = psc.tile([O, 512], F32, tag="conv")
                        rhsA = xtq[0:96, off0:off0 + 520] \
                            .rearrange("c (i w) -> c i w", w=W130)[:, :, 0:128]
                        rhsB = xtq[0:48, off0 + 260:off0 + 780] \
                            .rearrange("c (i w) -> c i w", w=W130)[:, :, 0:128]
                        nc.tensor.matmul(ps_o[:].rearrange("o (i t) -> o i t", t=128),
                                         wk_t[0:96, 0:O], rhsA, start=True, stop=False)
                        nc.tensor.matmul(ps_o[:].rearrange("o (i t) -> o i t", t=128),
                                         wk_t[0:48, 32:32 + O], rhsB, start=False, stop=True)
                        if pp % 2 == 0:
                            nc.scalar.copy(out=ost[:, bass.ts(pp, 512)], in_=ps_o[:])
                        else:
                            nc.vector.tensor_copy(ost[:, bass.ts(pp, 512)], ps_o[:])
                    nc.sync.dma_start(out=out_part[:, bass.ts(q, 2048)], in_=ost[:])

    _split_excess_waits(nc)
    return nc


_NC_CACHE = {}


def _get_nc():
    if "nc" not in _NC_CACHE:
        _NC_CACHE["nc"] = _build()
    return _NC_CACHE["nc"]


def _consts():
    if "consts" in _NC_CACHE:
        return _NC_CACHE["consts"]
    import ml_dtypes
    r = np.arange(N1)
    q = np.arange(N2)
    a100 = 2 * np.pi * np.outer(r, r) / N1
    a169 = 2 * np.pi * np.outer(q, q) / N2
    t1 = 2 * np.pi * np.outer(r, q) / N       # [r, b]
    c169c, c169s = np.cos(a169), np.sin(a169)

    def put(blob, r0, c0, v):
        blob[r0:r0 + v.shape[0], c0:c0 + v.shape[1]] = v

    blobA = np.zeros((128, CA), dtype=np.float32)
    put(blobA, 0, 0, np.cos(a100))
    put(blobA, 0, 100, -np.sin(a100))
    put(blobA, 0, 200, np.tile(np.cos(t1), (1, 2)))
    put(blobA, 0, 538, np.tile(np.sin(t1), (1, 2)))
    put(blobA, 0, 876, np.tile(-np.sin(t1), (1, 2)))
    put(blobA, 0, 1214, np.eye(128))
    put(blobA, 0, 1342, c169c[0:128])
    put(blobA, 0, 1511, c169s[0:128])
    # inverse 169-DFT half-spectrum rows, conjugate doubling (x2) folded in.
    # (drops the exact w=1 at DC(0,0) and the q=84 row: ~3e-4 rel error)
    put(blobA, 0, 1680, 2.0 * c169c[0:QM])
    put(blobA, 0, 1849, 2.0 * c169s[0:QM])
    put(blobA, 0, 2018, -2.0 * c169s[0:QM])
    put(blobA, 0, 2187, c169c[128:N2])
    put(blobA, 0, 2356, c169s[128:N2])
    put(blobA, 0, 2525, -c169s[0:128])
    put(blobA, 0, 2694, -c169s[128:N2])

    blobH = np.zeros((128, CH), dtype=np.float32)
    put(blobH, 0, 0, np.tile(np.cos(t1), (1, S)))
    put(blobH, 0, 2704, np.tile(np.sin(t1), (1, S)))
    put(blobH, 0, 5408, -np.cos(a100) / N)
    put(blobH, 0, 5508, np.sin(a100) / N)
    put(blobH, 0, 5608, np.eye(128))
    put(blobH, 0, 5736, c169c[0:128])
    put(blobH, 0, 5905, -c169s[0:128])
    put(blobH, 0, 6074, c169c[128:N2])
    put(blobH, 0, 6243, -c169s[128:N2])

    cc = {"blobA": blobA,
          "blobH": blobH.astype(ml_dtypes.bfloat16)}
    _NC_CACHE["consts"] = cc
    return cc


def kernel(x, weight, bias):
    x = np.asarray(x, dtype=np.float32)
    weight = np.asarray(weight, dtype=np.float32)
    bias = np.asarray(bias, dtype=np.float32)
    nc = _get_nc()
    cc = _consts()

    xp = np.pad(x, ((0, 0), (0, 0), (1, 1), (1, 1)))          # (4,32,130,130)
    # [a, (c, b)] layout of the flat 16900 signal, per batch
    x0s = [np.ascontiguousarray(
        xp[b].reshape(C, N).reshape(C, N1, N2).transpose(1, 0, 2).reshape(N1, C * N2))
        for b in range(B)]

    in_maps = []
    for core in range(8):
        b, h = core // 2, core % 2
        c0 = h * S
        # wk[:, 0:32]: rows (r in {0,1}, s, c) -> weight[o, c, r, s]
        # wk[0:48, 32:64]: rows (s, c) -> weight[o, c, 2, s]  (tap row r=2)
        wkm = np.zeros((96, 64), dtype=np.float32)
        for rr in range(2):
            for s in range(3):
                wkm[rr * 48 + s * 16:rr * 48 + s * 16 + 16, 0:32] = \
                    weight[:, c0:c0 + S, rr, s].T
        for s in range(3):
            wkm[s * 16:s * 16 + 16, 32:64] = weight[:, c0:c0 + S, 2, s].T
        m = {"x0": np.ascontiguousarray(x0s[b][:, c0 * N2:(c0 + S) * N2]),
             "wk": wkm.astype(ml_dtypes.bfloat16)}
        m.update(cc)
        in_maps.append(m)

    res = run_bass_kernel_spmd(nc, in_maps, core_ids=list(range(8)))

    out = np.empty((B, O, 128, 128), dtype=np.float32)
    for b in range(B):
        acc = res.results[2 * b]["out_part"] + res.results[2 * b + 1]["out_part"]
        out[b] = acc.reshape(O, 128, 128)
    out += bias[None, :, None, None]
    return out


# revision 98
# speedup vs baseline: 1.0569x; 1.0121x over previous
"""FFTConv1d-with-threshold kernel for Trainium2, 8 NeuronCores.

Math: the reference (flat 16900-pt FFT -> prune coeffs with |Re|<0.01 ->
multiply by kernel FFT -> iFFT -> roll -> channel-sum -> slice) is
algebraically a standard 3x3 pad-1 conv2d applied to (xp - delta), where
delta is the inverse FFT of the pruned (below-threshold) coefficients.

Sharding: core = (batch b, input-channel half). Each core runs the FFT/
prune/inverse pipeline for its 16 channels and a partial 3x3 conv over
all 32 out-channels; the host sums the two partials per batch (+bias).

Device pipeline per core: 16900-pt FFT via Cooley-Tukey (100 x 169) as
fp32r PE matmuls, twiddle1 on DVE/Pool (fp32), prune mask, inverse
(fp32r/bf16) with twiddle2 applied post-transpose in bf16, subtract,
DRAM regather into 6 (tap-row, shift) replicated conv operand groups,
3x3 conv as 2 PSUM-accumulated matmuls per 4-row block.
"""

import ml_dtypes
import numpy as np

import bass_rust
import concourse.bass as bass
import concourse.mybir as mybir
from concourse.bass_utils import run_bass_kernel_spmd
from concourse.tile import TileContext

F32 = mybir.dt.float32
F32R = mybir.dt.float32r
BF16 = mybir.dt.bfloat16

N1, N2, N = 100, 169, 16900
W130 = 130
B, C, O = 4, 32, 32
S = C // 2           # channels per core (channel-split sharding)
THRESH = 0.01
FS1 = 338            # stage-1 F chunk: 2 channels x 169
NF1 = (S * N2) // FS1
FS2 = 400            # stage-2 F chunk: 4 channels x 100
NF2 = (S * N1) // FS2
QM = 85              # kept half-spectrum rows of the 169-DFT
QCH = [(0, 128), (128, 41)]
QS2 = 4736           # conv quarter span (cols), base 4160*g
XDPAD = 17536        # padded xdram row length (max read 4160*3+262+4735)
CA = 2863            # packed fp32r const blob cols
CH = 6412            # packed bf16 const blob cols


def _split_excess_waits(nc):
    # This walrus build accepts 1 sync-wait slot per instruction; Tile can
    # attach several. Move extras onto nofuse NOPs on the same engine.
    for f in nc.m.functions:
        for blk in f.blocks:
            insts = blk.instructions
            changed = False
            new_list = []
            for inst in insts:
                si = inst.sync_info
                if si is not None and len(si.on_wait) > 1:
                    waits = list(si.on_wait)
                    extra, keep = waits[:-1], waits[-1:]
                    for k, w in enumerate(extra):
                        new_list.append(bass_rust.InstNoOp(
                            name=f"{inst.name}-ws{k}",
                            engine=inst.engine,
                            ins=[], outs=[], bass_nofuse=True,
                            sync_info=bass_rust.SyncInfo(on_wait=[w], on_update=[]),
                        ))
                    inst.sync_info = bass_rust.SyncInfo(
                        on_wait=keep, on_update=list(si.on_update))
                    changed = True
                new_list.append(inst)
            if changed:
                blk.instructions = new_list


def _build():
    nc = bass.Bass("TRN2")
    x0 = nc.dram_tensor("x0", [N1, S * N2], F32R, kind="ExternalInput")
    wk = nc.dram_tensor("wk", [96, 64], BF16, kind="ExternalInput")
    blobA_d = nc.dram_tensor("blobA", [128, CA], F32R, kind="ExternalInput")
    blobH_d = nc.dram_tensor("blobH", [128, CH], BF16, kind="ExternalInput")
    out_part = nc.dram_tensor("out_part", [O, 128 * 128], F32, kind="ExternalOutput")
    xdram = nc.dram_tensor("xdram", [S, XDPAD], BF16)

    with TileContext(nc) as tc:
        with tc.tile_pool(name="const", bufs=1) as cst, \
             tc.tile_pool(name="big", bufs=1) as big, \
             tc.tile_pool(name="chunk", bufs=4) as chk:

            # packed constant blobs: slice map must match _consts()
            blobA = cst.tile([128, CA], F32R, tag="blobA", name="blobA")
            blobH = cst.tile([128, CH], BF16, tag="blobH", name="blobH")
            ct = {
                "c100c": blobA[0:N1, 0:100],
                "c100sn": blobA[0:N1, 100:200],
                "t1c2": blobA[0:N1, 200:538],
                "t1s2": blobA[0:N1, 538:876],
                "t1sn2": blobA[0:N1, 876:1214],
                "c169c": {0: blobA[0:128, 1342:1511], 128: blobA[0:41, 2187:2356]},
                "c169s": {0: blobA[0:128, 1511:1680], 128: blobA[0:41, 2356:2525]},
                "c169sn": {0: blobA[0:128, 2525:2694], 128: blobA[0:41, 2694:2863]},
                "c169cr": blobA[0:QM, 1680:1849],
                "c169sr": blobA[0:QM, 1849:2018],
                "c169snr": blobA[0:QM, 2018:2187],
                "w2c": blobH[0:N1, 0:2704],
                "w2s": blobH[0:N1, 2704:5408],
                "cinvc": blobH[0:N1, 5408:5508],
                "cinvsn": blobH[0:N1, 5508:5608],
                "c169ch": {0: blobH[0:128, 5736:5905], 128: blobH[0:41, 6074:6243]},
                "c169snh": {0: blobH[0:128, 5905:6074], 128: blobH[0:41, 6243:6412]},
            }
            ident = blobA[0:128, 1214:1342]
            identh = blobH[0:128, 5608:5736]
            wk_t = cst.tile([96, 64], BF16, tag="wk")

            x0t = big.tile([N1, S * N2], F32R, tag="x0")
            # load order = first-use order: x0 chunk 0, stage-1 consts,
            # rest of x0, stage-2/inverse consts, tw2 tables, weights
            nc.sync.dma_start(out=x0t[:, bass.ts(0, FS1)], in_=x0[:, bass.ts(0, FS1)])
            nc.sync.dma_start(out=blobA[0:N1, 0:200], in_=blobA_d[0:N1, 0:200])
            nc.sync.dma_start(out=blobA[0:N1, 200:1214], in_=blobA_d[0:N1, 200:1214])
            for f in range(1, NF1):
                sl = bass.ts(f, FS1)
                nc.sync.dma_start(out=x0t[:, sl], in_=x0[:, sl])
            nc.sync.dma_start(out=blobA[:, 1214:CA], in_=blobA_d[:, 1214:CA])
            nc.sync.dma_start(out=blobH[:, 5408:CH], in_=blobH_d[:, 5408:CH])
            nc.sync.dma_start(out=blobH[0:N1, 0:5408], in_=blobH_d[0:N1, 0:5408])
            nc.sync.dma_start(out=wk_t[:], in_=wk[:])
            xtr = big.tile([N1, S * N2], BF16, tag="xtr", name="xtr")
            # conv operand groups (tap-row r in {0,1}, shift s):
            # partition 16*(3r+s)+c col (g,k): x[c, 4160*g + 130r + s + k]
            xtq = big.tile([96, 4 * QS2], BF16, tag="xtq", name="xtq")

            htwtp_cm = tc.tile_pool(name="htwtp", bufs=1)
            htwtp = htwtp_cm.__enter__()
            # +676 col pad so batched T2 copies can use a rearrange window
            htwt_re = htwtp.tile([N1, S * N2 + 676], BF16, tag="htwt_re")
            htwt_im = htwtp.tile([N1, S * N2 + 676], BF16, tag="htwt_im")
            gttp_cm = tc.tile_pool(name="gttp", bufs=1)
            gttp = gttp_cm.__enter__()
            gtt_re_a = gttp.tile([128, S * N1], F32R, tag="gtt_re_a")
            gtt_re_b = gttp.tile([41, S * N1], F32R, tag="gtt_re_b")
            gtt_im_a = gttp.tile([128, S * N1], F32R, tag="gtt_im_a")
            gtt_im_b = gttp.tile([41, S * N1], F32R, tag="gtt_im_b")

            # ---------- FWD stage 1 + twiddle1 + T1 ----------
            with tc.tile_pool(name="ps1", bufs=2, space="PSUM") as ps1, \
                 tc.tile_pool(name="pt1", bufs=2, space="PSUM") as pt1:
                for f1 in range(NF1):
                    sl = bass.ts(f1, FS1)
                    ps_re = ps1.tile([N1, FS1], F32, tag="s1re")
                    ps_im = ps1.tile([N1, FS1], F32, tag="s1im")
                    nc.tensor.matmul(ps_re[:], ct["c100c"][:], x0t[:, sl], start=True, stop=True)
                    nc.tensor.matmul(ps_im[:], ct["c100sn"][:], x0t[:, sl], start=True, stop=True)
                    g_re = chk.tile([N1, FS1], F32R, tag="gt_re")
                    g_im = chk.tile([N1, FS1], F32R, tag="gt_im")
                    tmp1 = chk.tile([N1, FS1], F32, tag="tw1tmp")
                    tmp2 = chk.tile([N1, FS1], F32, tag="tw1tmp2")
                    tmp3 = chk.tile([N1, FS1], F32, tag="tw1tmp3")
                    tmp4 = chk.tile([N1, FS1], F32, tag="tw1tmp4")
                    # Gt_re = Gre*t1c + Gim*t1s ; Gt_im = Gim*t1c - Gre*t1s
                    nc.vector.tensor_mul(out=tmp1[:], in0=ps_re[:], in1=ct["t1c2"][:])
                    nc.vector.tensor_mul(out=tmp2[:], in0=ps_im[:], in1=ct["t1s2"][:])
                    nc.gpsimd.tensor_add(out=g_re[:], in0=tmp1[:], in1=tmp2[:])
                    nc.vector.tensor_mul(out=tmp3[:], in0=ps_im[:], in1=ct["t1c2"][:])
                    nc.vector.tensor_mul(out=tmp4[:], in0=ps_re[:], in1=ct["t1sn2"][:])
                    nc.gpsimd.tensor_add(out=g_im[:], in0=tmp3[:], in1=tmp4[:])
                    w2 = bass.ds(2 * f1 * N1, 2 * N1)
                    for g_src, dst_a, dst_b in (
                            (g_re, gtt_re_a, gtt_re_b),
                            (g_im, gtt_im_a, gtt_im_b)):
                        pt = pt1.tile([128, 512], F32R, tag="ptx")
                        for j in range(2):
                            nc.tensor.transpose(pt[0:128, bass.ts(j, N1)], g_src[:, bass.ds(j * N2, 128)], ident[0:N1, 0:N1])
                            nc.tensor.transpose(pt[0:41, bass.ds(200 + j * N1, N1)], g_src[:, bass.ds(j * N2 + 128, 41)], ident[0:N1, 0:N1])
                        nc.scalar.copy(out=dst_a[:, w2], in_=pt[0:128, 0:200])
                        nc.scalar.copy(out=dst_b[:, w2], in_=pt[0:41, 200:400])

            # ---- FWD stage 2 + mask + INV169 + T2 + tw2 + INV100 + store ----
            with tc.tile_pool(name="ps2", bufs=1, space="PSUM") as ps2, \
                 tc.tile_pool(name="pi1", bufs=2, space="PSUM") as pi1, \
                 tc.tile_pool(name="pt2", bufs=2, space="PSUM") as pt2, \
                 tc.tile_pool(name="ps3", bufs=1, space="PSUM") as ps3, \
                 tc.tile_pool(name="twp", bufs=3) as twp:
                for f in range(NF2):
                    sl = bass.ts(f, FS2)
                    ps_xre = ps2.tile([128, FS2], F32, tag="s2re")
                    ps_xim = ps2.tile([128, FS2], F32, tag="s2im")
                    lc_a = ct["c169c"][0][:, 0:QM]
                    lc_b = ct["c169c"][128][:, 0:QM]
                    ls_a = ct["c169s"][0][:, 0:QM]
                    ls_b = ct["c169s"][128][:, 0:QM]
                    # Xre = Cc.T@GtTre + Cs.T@GtTim   (fp32r)
                    nc.tensor.matmul(ps_xre[0:QM], lc_a, gtt_re_a[:, sl], start=True, stop=False)
                    nc.tensor.matmul(ps_xre[0:QM], lc_b, gtt_re_b[:, sl], start=False, stop=False)
                    nc.tensor.matmul(ps_xre[0:QM], ls_a, gtt_im_a[:, sl], start=False, stop=False)
                    nc.tensor.matmul(ps_xre[0:QM], ls_b, gtt_im_b[:, sl], start=False, stop=True)
                    # Xim = Cc.T@GtTim - Cs.T@GtTre   (fp32r)
                    nc.tensor.matmul(ps_xim[0:QM], ct["c169sn"][0][:, 0:QM], gtt_re_a[:, sl], start=True, stop=False)
                    nc.tensor.matmul(ps_xim[0:QM], ct["c169sn"][128][:, 0:QM], gtt_re_b[:, sl], start=False, stop=False)
                    nc.tensor.matmul(ps_xim[0:QM], ct["c169c"][0][:, 0:QM], gtt_im_a[:, sl], start=False, stop=False)
                    nc.tensor.matmul(ps_xim[0:QM], ct["c169c"][128][:, 0:QM], gtt_im_b[:, sl], start=False, stop=True)
                    ps_xre = ps_xre[0:QM]
                    ps_xim = ps_xim[0:QM]
                    pm = chk.tile([QM, FS2], F32, tag="pm")
                    nc.scalar.activation(pm[:], ps_xre[:],
                                         mybir.ActivationFunctionType.Abs)
                    zr = chk.tile([QM, FS2], F32R, tag="zre")
                    zi = chk.tile([QM, FS2], F32R, tag="zim")
                    # z = (|Xre| < t) * X   (fused mask-and-apply)
                    nc.vector.scalar_tensor_tensor(
                        out=zr[:], in0=pm[:], scalar=THRESH, in1=ps_xre[:],
                        op0=mybir.AluOpType.is_lt, op1=mybir.AluOpType.mult)
                    nc.vector.scalar_tensor_tensor(
                        out=zi[:], in0=pm[:], scalar=THRESH, in1=ps_xim[:],
                        op0=mybir.AluOpType.is_lt, op1=mybir.AluOpType.mult)

                    for (b0, bn) in QCH:
                        ps_hre = pi1.tile([128, FS2], F32, tag="i1re")
                        ps_him = pi1.tile([128, FS2], F32, tag="i1im", bufs=1)
                        lc = ct["c169cr"][:, bass.ds(b0, bn)]
                        ls = ct["c169sr"][:, bass.ds(b0, bn)]
                        lsn = ct["c169snr"][:, bass.ds(b0, bn)]
                        # Hre = Cc.T@Zre - Cs.T@Zim ; Him = Cs.T@Zre + Cc.T@Zim
                        nc.tensor.matmul(ps_hre[0:bn], lc, zr[:], start=True, stop=False)
                        nc.tensor.matmul(ps_hre[0:bn], lsn, zi[:], start=False, stop=True)
                        nc.tensor.matmul(ps_him[0:bn], ls, zr[:], start=True, stop=False)
                        nc.tensor.matmul(ps_him[0:bn], lc, zi[:], start=False, stop=True)
                        h_re = chk.tile([128, FS2], BF16, tag="h_re")
                        h_im = chk.tile([128, FS2], BF16, tag="h_im")
                        nc.scalar.copy(out=h_re[0:bn], in_=ps_hre[0:bn])
                        nc.vector.tensor_copy(h_im[0:bn], ps_him[0:bn])
                        for src, dst in ((h_re, htwt_re), (h_im, htwt_im)):
                            pt = pt2.tile([N1, 512], BF16, tag="t2p")
                            for j in range(4):
                                nc.tensor.transpose(pt[:, bass.ds(j * 128, bn)], src[0:bn, bass.ts(j, N1)], identh[0:bn, 0:bn])
                            nc.scalar.copy(
                                out=dst[:, bass.ds(4 * f * N2 + b0, 676)]
                                    .rearrange("p (j b) -> p j b", b=N2)[:, :, 0:bn],
                                in_=pt[:].rearrange("p (j b) -> p j b", b=128)[:, :, 0:bn])

                    # twiddle2 for this chunk's 4 channels, [k1,(c,n2)] bf16:
                    # Htw = H * (w2c + i*w2s): re = re*c - im*s ; im = re*s + im*c
                    us = bass.ds(f * 4 * N2, 4 * N2)
                    ta = twp.tile([N1, 4 * N2], BF16, tag="tta", name="tta")
                    tb = twp.tile([N1, 4 * N2], BF16, tag="ttb", name="ttb")
                    nc.gpsimd.tensor_mul(out=ta[:], in0=htwt_re[:, us], in1=ct["w2s"][:, us])
                    nc.vector.tensor_mul(out=tb[:], in0=htwt_im[:, us], in1=ct["w2s"][:, us])
                    nc.vector.tensor_mul(out=htwt_re[:, us], in0=htwt_re[:, us], in1=ct["w2c"][:, us])
                    nc.vector.tensor_mul(out=htwt_im[:, us], in0=htwt_im[:, us], in1=ct["w2c"][:, us])
                    nc.vector.tensor_sub(out=htwt_re[:, us], in0=htwt_re[:, us], in1=tb[:])
                    nc.vector.tensor_add(out=htwt_im[:, us], in0=htwt_im[:, us], in1=ta[:])

                    # INV stage 2 fused with subtract: psum = I@x0 - cinv@Htw
                    # (cinvc/cinvsn are pre-negated on the host)
                    for f1 in (2 * f, 2 * f + 1):
                        sl1 = bass.ts(f1, FS1)
                        ps_d = ps3.tile([N1, FS1], F32, tag="dlt")
                        nc.tensor.matmul(ps_d[:], ident[0:N1, 0:N1], x0t[:, sl1], start=True, stop=False)
                        nc.tensor.matmul(ps_d[:], ct["cinvc"][:], htwt_re[:, sl1], start=False, stop=False)
                        nc.tensor.matmul(ps_d[:], ct["cinvsn"][:], htwt_im[:, sl1], start=False, stop=True)
                        nc.scalar.copy(out=xtr[:, sl1], in_=ps_d[:])
                    # flatten this chunk's 4 channels to DRAM [c, 16900] (bf16)
                    nc.gpsimd.dma_start(
                        out=xdram[bass.ds(4 * f, 4), 0:N]
                            .rearrange("c (a b) -> a c b", b=N2),
                        in_=xtr[:, bass.ds(f * 4 * N2, 4 * N2)])
                    # conv operand sub-gathers for this chunk's 4 channels
                    for r in range(2):
                        for s in range(3):
                            grp = r * 3 + s
                            src = bass.AP(
                                tensor=xdram, offset=4 * f * XDPAD + 130 * r + s,
                                ap=[[XDPAD, 4], [4160, 4], [1, QS2]])
                            nc.sync.dma_start(
                                out=xtq[bass.ds(16 * grp + 4 * f, 4), :]
                                    .rearrange("c (g k) -> c g k", k=QS2),
                                in_=src)

            gttp_cm.__exit__(None, None, None)
            htwtp_cm.__exit__(None, None, None)

            # ---------- conv 3x3 valid: 2 matmuls per 4-row block ----------
            with tc.tile_pool(name="psc", bufs=6, space="PSUM") as psc, \
                 tc.tile_pool(name="pswm", bufs=1, space="PSUM") as pswm, \
                 tc.tile_pool(name="ostp", bufs=4) as ostp:
                # keep the PE clock ramped through the gather gap: a chain of
                # throwaway matmuls on resident data (result never read)
                ps_w = pswm.tile([32, 512], F32, tag="warm", name="ps_w")
                for _ in range(29):
                    nc.tensor.matmul(ps_w[:], ident[0:N1, 0:32],
                                     x0t[:, 0:512], start=True, stop=True)
                for q in range(8):
                    ost = ostp.tile([O, 2048], F32, tag="ost", name="ost")
                    for pp in range(4):
                        p = 4 * q + pp
                        g, lp = p // 8, p % 8
                        off0 = 520 * lp + QS2 * g
                        ps_o = psc.tile([O, 512], F32, tag="conv")
                        rhsA = xtq[0:96, off0:off0 + 520] \
                            .rearrange("c (i w) -> c i w", w=W130)[:, :, 0:128]
                        rhsB = xtq[0:48, off0 + 260:off0 + 780] \
                            .rearrange("c (i w) -> c i w", w=W130)[:, :, 0:128]
                        nc.tensor.matmul(ps_o[:].rearrange("o (i t) -> o i t", t=128),
                                         wk_t[0:96, 0:O], rhsA, start=True, stop=False)
                        nc.tensor.matmul(ps_o[:].rearrange("o (i t) -> o i t", t=128),
                                         wk_t[0:48, 32:32 + O], rhsB, start=False, stop=True)
                        if pp % 2 == 0:
                            nc.scalar.copy(out=ost[:, bass.ts(pp, 512)], in_=ps_o[:])
                        else:
                            nc.vector.tensor_copy(ost[:, bass.ts(pp, 512)], ps_o[:])
                    nc.sync.dma_start(out=out_part[:, bass.ts(q, 2048)], in_=ost[:])

    _split_excess_waits(nc)
    return nc


_NC_CACHE = {}


def _get_nc():
    if "nc" not in _NC_CACHE:
        _NC_CACHE["nc"] = _build()
    return _NC_CACHE["nc"]


def _consts():
    if "consts" in _NC_CACHE:
        return _NC_CACHE["consts"]
    import ml_dtypes
    r = np.arange(N1)
    q = np.arange(N2)
    a100 = 2 * np.pi * np.outer(r, r) / N1
    a169 = 2 * np.pi * np.outer(q, q) / N2
    t1 = 2 * np.pi * np.outer(r, q) / N       # [r, b]
    c169c, c169s = np.cos(a169), np.sin(a169)

    def put(blob, r0, c0, v):
        blob[r0:r0 + v.shape[0], c0:c0 + v.shape[1]] = v

    blobA = np.zeros((128, CA), dtype=np.float32)
    put(blobA, 0, 0, np.cos(a100))
    put(blobA, 0, 100, -np.sin(a100))
    put(blobA, 0, 200, np.tile(np.cos(t1), (1, 2)))
    put(blobA, 0, 538, np.tile(np.sin(t1), (1, 2)))
    put(blobA, 0, 876, np.tile(-np.sin(t1), (1, 2)))
    put(blobA, 0, 1214, np.eye(128))
    put(blobA, 0, 1342, c169c[0:128])
    put(blobA, 0, 1511, c169s[0:128])
    # inverse 169-DFT half-spectrum rows, conjugate doubling (x2) folded in.
    # (drops the exact w=1 at DC(0,0) and the q=84 row: ~3e-4 rel error)
    put(blobA, 0, 1680, 2.0 * c169c[0:QM])
    put(blobA, 0, 1849, 2.0 * c169s[0:QM])
    put(blobA, 0, 2018, -2.0 * c169s[0:QM])
    put(blobA, 0, 2187, c169c[128:N2])
    put(blobA, 0, 2356, c169s[128:N2])
    put(blobA, 0, 2525, -c169s[0:128])
    put(blobA, 0, 2694, -c169s[128:N2])

    blobH = np.zeros((128, CH), dtype=np.float32)
    put(blobH, 0, 0, np.tile(np.cos(t1), (1, S)))
    put(blobH, 0, 2704, np.tile(np.sin(t1), (1, S)))
    put(blobH, 0, 5408, -np.cos(a100) / N)
    put(blobH, 0, 5508, np.sin(a100) / N)
    put(blobH, 0, 5608, np.eye(128))
    put(blobH, 0, 5736, c169c[0:128])
    put(blobH, 0, 5905, -c169s[0:128])
    put(blobH, 0, 6074, c169c[128:N2])
    put(blobH, 0, 6243, -c169s[128:N2])

    cc = {"blobA": blobA,
          "blobH": blobH.astype(ml_dtypes.bfloat16)}
    _NC_CACHE["consts"] = cc
    return cc


def kernel(x, weight, bias):
    x = np.asarray(x, dtype=np.float32)
    weight = np.asarray(weight, dtype=np.float32)
    bias = np.asarray(bias, dtype=np.float32)
    nc = _get_nc()
    cc = _consts()

    xp = np.pad(x, ((0, 0), (0, 0), (1, 1), (1, 1)))          # (4,32,130,130)
    # [a, (c, b)] layout of the flat 16900 signal, per batch
    x0s = [np.ascontiguousarray(
        xp[b].reshape(C, N).reshape(C, N1, N2).transpose(1, 0, 2).reshape(N1, C * N2))
        for b in range(B)]

    in_maps = []
    for core in range(8):
        b, h = core // 2, core % 2
        c0 = h * S
        # wk[:, 0:32]: rows (r in {0,1}, s, c) -> weight[o, c, r, s]
        # wk[0:48, 32:64]: rows (s, c) -> weight[o, c, 2, s]  (tap row r=2)
        wkm = np.zeros((96, 64), dtype=np.float32)
        for rr in range(2):
            for s in range(3):
                wkm[rr * 48 + s * 16:rr * 48 + s * 16 + 16, 0:32] = \
                    weight[:, c0:c0 + S, rr, s].T
        for s in range(3):
            wkm[s * 16:s * 16 + 16, 32:64] = weight[:, c0:c0 + S, 2, s].T
        m = {"x0": np.ascontiguousarray(x0s[b][:, c0 * N2:(c0 + S) * N2]),
             "wk": wkm.astype(ml_dtypes.bfloat16)}
        m.update(cc)
        in_maps.append(m)

    res = run_bass_kernel_spmd(nc, in_maps, core_ids=list(range(8)))

    out = np.empty((B, O, 128, 128), dtype=np.float32)
    for b in range(B):
        acc = res.results[2 * b]["out_part"] + res.results[2 * b + 1]["out_part"]
        out[b] = acc.reshape(O, 128, 128)
    out += bias[None, :, None, None]
    return out
